# revision 6
# baseline (speedup 1.0000x reference)
"""HTSK fuzzy-system kernel for Trainium2 (Bass/Tile), 8-core data-parallel.

Math (per batch row b):
  S     = H/sigma^2 + EPS                          (D,R)
  m     = mean_d(-(X_bd - C_dr)^2 * S_dr)          (B,R)
        = X^2 @ (-S/D) + X @ (2*S*C/D) + K2        (matmul expansion)
  e     = exp(m)            (unnormalized softmax; m is bounded, no max needed)
  out   = (1/sum_r e) * ( sum_r e_br * G_bro  +  e @ (W2 + 1 b^T) )
  G     = X @ Wt,  Wt[d, o*R+r] = W[r*D+d, o]      (B, O*R)  << o-major!

o-major G layout makes the e-broadcast multiply innermost-contiguous in r,
so DVE runs it in 2x_1P mode. When sigmas are uniform the X^2 term is
constant over r and cancels in the softmax (normalizer absorbs it), so the
A-matmuls are dropped entirely. sum_r e comes free from a ones column
appended to W2.

Engine split per 128-row tile (aiming ~7.5us each, fully overlapped):
  PE      : 32 G-matmuls (N=512, bf16) + logits + e-transpose + out2
  ScalarE : drains G chunks 0-6 PSUM fp32 -> SBUF bf16, exp, final scale
  DVE     : chunk-7 drain+mult from PSUM, one 2x multiply for chunks 0-6,
            tree levels 2-7, reciprocal, final add
  SDMA    : tree level 1 via SBUF->SBUF accum_op=add (CCE inline adder),
            issued from the otherwise-idle GpSimd queue (SWDGE)

Sharding: batch B=4096 split 512 rows per core; weights replicated.
All constants ship in one packed [128, x] DMA blob per core.
"""
import sys
import types

import numpy as np

sys.path.insert(0, "/opt/trn_rl_repo")

# NTFF profile-hook registry: trn_boot sets it at jax init, concourse
# bass_utils reads it when trace=True. The container's antenv package lacks
# this submodule, so provide it before anything imports jax/concourse.
if "antenv.axon_hooks" not in sys.modules:
    _ah = types.ModuleType("antenv.axon_hooks")
    _ah._hook = None

    def _set_hook(hook):
        _ah._hook = hook

    def _get_hook():
        return _ah._hook

    _ah.set_axon_ntff_profile_hook = _set_hook
    _ah.get_axon_ntff_profile_hook = _get_hook
    sys.modules["antenv.axon_hooks"] = _ah

import ml_dtypes  # noqa: E402
import concourse.bass as bass  # noqa: E402
import concourse.bacc as bacc  # noqa: E402
import concourse.tile as tile  # noqa: E402
from contextlib import ExitStack  # noqa: E402
from concourse import mybir  # noqa: E402
from concourse import bass_utils  # noqa: E402
from concourse.masks import make_identity  # noqa: E402

H = 0.5
EPS = 1e-8
B, D, R, O = 4096, 256, 128, 64
NCORES = 8
BL = B // NCORES          # 512 batch rows per core
NT = BL // 128            # 4 partition tiles per core
RO = R * O                # 8192 G columns per row, o-major: col = o*R + r
F32 = mybir.dt.float32
BF16 = mybir.dt.bfloat16
EXPF = mybir.ActivationFunctionType.Exp
ADD = mybir.AluOpType.add

# const blob column offsets (bf16 elements)
OBM, OK2, OW2, OXT = 0, 256, 384, 520
NC_BASE = OXT + 1024          # 1544
OA = NC_BASE                  # A appended when sigmas are non-uniform

_CACHE = {}
LAST_RESULT = None
TRACE = False
TRACE_DIR = "/root/problem/work/trace_out"
import os
L1_DMA = os.environ.get("L1_DMA", "1") == "1"   # level-1 tree on SDMA vs DVE


def _build(use_a: bool):
    ncols = NC_BASE + (256 if use_a else 0)
    nc = bacc.Bacc("TRN2", target_bir_lowering=False, debug=False)
    CONST = nc.dram_tensor("CONST", [128, ncols], BF16, kind="ExternalInput")
    WT = nc.dram_tensor("WT", [D, RO], BF16, kind="ExternalInput")
    out = nc.dram_tensor("out", [BL, O], F32, kind="ExternalOutput")

    with tile.TileContext(nc) as tc, ExitStack() as ctx:
        consts = ctx.enter_context(tc.tile_pool(name="consts", bufs=1))
        work = ctx.enter_context(tc.tile_pool(name="work", bufs=2))
        gsbp = ctx.enter_context(tc.tile_pool(name="gsb", bufs=2))
        gmp = ctx.enter_context(tc.tile_pool(name="gm", bufs=2))
        treep = ctx.enter_context(tc.tile_pool(name="tree", bufs=2))
        ps_m = ctx.enter_context(tc.tile_pool(name="ps_m", bufs=1, space="PSUM"))
        ps_e = ctx.enter_context(tc.tile_pool(name="ps_e", bufs=2, space="PSUM"))
        ps_o = ctx.enter_context(tc.tile_pool(name="ps_o", bufs=1, space="PSUM"))
        ps_g = ctx.enter_context(tc.tile_pool(name="ps_g", bufs=2, space="PSUM"))

        # ---- one packed const DMA, then Wt chunk-tiles stream behind ----
        conc = consts.tile([128, ncols], BF16, tag="conc")
        nc.sync.dma_start(out=conc[:, :], in_=CONST[:, :])
        bm_sb = conc[:, OBM:OBM + 256].rearrange("p (c r) -> p c r", r=R)
        k2_sb = conc[0:1, OK2:OK2 + R]
        w2b_sb = conc[:, OW2:OW2 + O + 1]          # [R, 65]: W2+b ++ ones col
        xTv = conc[:, OXT:OXT + 1024].rearrange("p (c j) -> p c j", j=BL)
        if use_a:
            a_sb = conc[:, OA:OA + 256].rearrange("p (c r) -> p c r", r=R)
        identB = consts.tile([128, 128], BF16, tag="idb")
        make_identity(nc, identB)
        ones_sb = consts.tile([1, 128], BF16, tag="ones")
        nc.vector.memset(ones_sb, 1.0)
        # Wt: one [128, 1024] tile per (c, chunk-column), ordered by first use
        wt_sb = [[None] * 8, [None] * 8]
        for ch in range(8):
            for c in range(2):
                t_ = consts.tile([128, 1024], BF16, tag=f"wt{c}{ch}")
                nc.sync.dma_start(
                    out=t_[:, :], in_=WT[c * 128:(c + 1) * 128,
                                         ch * 1024:(ch + 1) * 1024]
                )
                wt_sb[c][ch] = t_
        if use_a:
            x2T = consts.tile([128, 2, BL], BF16, tag="x2T")
            for c in range(2):
                nc.scalar.square(x2T[:, c, :], xTv[:, c, :])

        for t in range(NT):
            bs = slice(t * 128, (t + 1) * 128)
            # ---- membership logits m = [X^2 @ A] + X @ Bm + 1*K2 ----
            m_ps = ps_m.tile([128, R], F32, tag="m")
            first = True
            if use_a:
                for c in range(2):
                    nc.tensor.matmul(m_ps, lhsT=x2T[:, c, bs], rhs=a_sb[:, c, :],
                                     start=first, stop=False)
                    first = False
            for c in range(2):
                nc.tensor.matmul(m_ps, lhsT=xTv[:, c, bs], rhs=bm_sb[:, c, :],
                                 start=first, stop=False)
                first = False
            nc.tensor.matmul(m_ps, lhsT=ones_sb, rhs=k2_sb, start=False, stop=True)

            # ---- unnormalized softmax weights: e = exp(m) ----
            e_bf = work.tile([128, R], BF16, tag="e")
            nc.scalar.activation(e_bf, m_ps, EXPF, bias=0.0, scale=1.0)

            # ---- out2 = e @ [W2+b | 1]: col 64 gives sum_r e for free ----
            eT_ps = ps_e.tile([128, 128], BF16, tag="eT")
            nc.tensor.transpose(eT_ps, e_bf, identB)
            eT_sb = work.tile([128, 128], BF16, tag="eTsb")
            nc.vector.tensor_copy(eT_sb, eT_ps)
            out2_ps = ps_o.tile([128, O + 1], F32, tag="out2")
            nc.tensor.matmul(out2_ps, lhsT=eT_sb, rhs=w2b_sb, start=True, stop=True)
            rs = work.tile([128, 1], F32, tag="rs")
            nc.vector.reciprocal(rs, out2_ps[:, O:O + 1])

            # ---- G = X @ Wt in o-major (8 chunks of 1024 cols = 8 o-blocks) ----
            gsb = gsbp.tile([128, 7168], BF16, tag="gsb")
            gm = gmp.tile([128, RO], BF16, tag="gm")
            gm3 = gm.rearrange("p (o r) -> p o r", r=R)
            gsb3 = gsb.rearrange("p (o r) -> p o r", r=R)

            def ebc(n):
                return e_bf.rearrange("p r -> p () r").broadcast_to((128, n, R))

            for ch in range(8):
                gt = ps_g.tile([128, 1024], F32, tag="g", name=f"g_{t}_{ch}")
                for c in range(2):
                    for h in range(2):
                        mm = nc.tensor.matmul(
                            gt[:, h * 512:(h + 1) * 512],
                            lhsT=xTv[:, c, bs],
                            rhs=wt_sb[c][ch][:, h * 512:(h + 1) * 512],
                            start=(c == 0), stop=(c == 1),
                        )
                        if h == 1:
                            # same stationary as h==0: drop the redundant
                            # LDWEIGHTS so the background weight buffer stays
                            # free for the next c's prefetch
                            mm.ins.ldweights = False
                if ch < 7:
                    # ScalarE evicts PSUM fp32 -> SBUF bf16
                    nc.scalar.copy(gsb[:, ch * 1024:(ch + 1) * 1024], gt)
                else:
                    # DVE drains + multiplies the last chunk straight from PSUM
                    gt3 = gt.rearrange("p (o r) -> p o r", r=R)
                    nc.vector.tensor_mul(gm3[:, 56:64, :], gt3, ebc(8))

            # ---- weighted by e: one 2x_1P DVE multiply (innermost r step-1) ----
            nc.vector.tensor_mul(gm3[:, 0:56, :], gsb3[:, 0:56, :], ebc(56))

            # ---- sum over r: level 1 on the DMA engines (CCE inline add),
            # in place: gm[:, :, 0:64] += gm[:, :, 64:128].  Last tile keeps
            # level 1 on DVE to shorten the epilogue chain. ----
            if L1_DMA and t < NT - 1:
                nc.gpsimd.dma_start(out=gm3[:, :, 0:64], in_=gm3[:, :, 64:128],
                                    accum_op=ADD)
                l2a, l2b = gm3[:, :, 0:32], gm3[:, :, 32:64]
            else:
                t1 = treep.tile([128, 64, 64], BF16, tag="t1")
                nc.vector.tensor_add(t1, gm3[:, :, 0:64], gm3[:, :, 64:128])
                l2a, l2b = t1[:, :, 0:32], t1[:, :, 32:64]
            t2 = treep.tile([128, 64, 32], BF16, tag="t2")
            nc.vector.tensor_add(t2, l2a, l2b)
            t3 = treep.tile([128, 64, 16], BF16, tag="t3")
            nc.vector.tensor_add(t3, t2[:, :, 0:16], t2[:, :, 16:32])
            t4 = treep.tile([128, 64, 8], BF16, tag="t4")
            nc.vector.tensor_add(t4, t3[:, :, 0:8], t3[:, :, 8:16])
            t5 = treep.tile([128, 64, 4], BF16, tag="t5")
            nc.vector.tensor_add(t5, t4[:, :, 0:4], t4[:, :, 4:8])
            t6 = treep.tile([128, 64, 2], BF16, tag="t6")
            nc.vector.tensor_add(t6, t5[:, :, 0:2], t5[:, :, 2:4])
            red = work.tile([128, 64, 1], BF16, tag="red")
            nc.vector.tensor_add(red, t6[:, :, 0:1], t6[:, :, 1:2])

            # ---- out = rs * (red + out2) ----
            osb = work.tile([128, O], F32, tag="osb")
            nc.vector.tensor_add(osb, red.rearrange("p o () -> p o"),
                                 out2_ps[:, 0:O])
            nc.scalar.mul(osb, osb, rs)
            nc.sync.dma_start(out=out[t * 128:(t + 1) * 128, :], in_=osb)

    nc.finalize()
    return nc


def _get_nc(use_a: bool):
    key = ("nc", use_a)
    if key not in _CACHE:
        _CACHE[key] = _build(use_a)
    return _CACHE[key]


def _host_prep(centers, sigmas, W, b):
    c64 = centers.astype(np.float64)
    S = (H / sigmas.astype(np.float64) ** 2) + EPS          # (D,R)
    use_a = not np.allclose(S, S.flat[0])
    bf = ml_dtypes.bfloat16
    ncols = NC_BASE + (256 if use_a else 0)
    CB = np.zeros((128, ncols), dtype=bf)
    Bm = (2.0 * S * c64 / D).astype(bf)                      # X coeff
    CB[:, OBM:OBM + 128] = Bm[0:128]
    CB[:, OBM + 128:OBM + 256] = Bm[128:256]
    K2 = (-(S * c64 * c64).sum(axis=0) / D).astype(bf)
    CB[0, OK2:OK2 + R] = K2
    W2b = np.concatenate(
        [W[D * R:].astype(np.float64) + b[None, :].astype(np.float64),
         np.ones((R, 1))], axis=1
    ).astype(bf)
    CB[:, OW2:OW2 + O + 1] = W2b
    if use_a:
        A = (-S / D).astype(bf)
        CB[:, OA:OA + 128] = A[0:128]
        CB[:, OA + 128:OA + 256] = A[128:256]
    W1 = W[: D * R].reshape(R, D, O)
    # o-major: Wt[d, o*R + r] = W1[r, d, o]
    Wt = np.ascontiguousarray(W1.transpose(1, 2, 0).reshape(D, RO)).astype(bf)
    return use_a, CB, Wt


def kernel(X, centers, sigmas, W, b):
    global LAST_RESULT
    X = np.asarray(X, dtype=np.float32)
    centers = np.asarray(centers, dtype=np.float32)
    sigmas = np.asarray(sigmas, dtype=np.float32)
    W = np.asarray(W, dtype=np.float32)
    b = np.asarray(b, dtype=np.float32)

    use_a, CB, Wt = _host_prep(centers, sigmas, W, b)
    Xb = X.astype(ml_dtypes.bfloat16)
    nc = _get_nc(use_a)
    in_maps = []
    for k in range(NCORES):
        cb = CB.copy()
        xt = Xb[k * BL:(k + 1) * BL].T                      # (D, BL)
        cb[:, OXT:OXT + BL] = xt[0:128]
        cb[:, OXT + BL:OXT + 1024] = xt[128:256]
        in_maps.append({"CONST": cb, "WT": Wt})
    kw = {}
    if TRACE:
        import shutil
        shutil.rmtree(TRACE_DIR, ignore_errors=True)
        kw = {"trace": True, "tmpdir": TRACE_DIR}
    res = bass_utils.run_bass_kernel_spmd(
        nc, in_maps, core_ids=list(range(NCORES)), **kw
    )
    LAST_RESULT = res
    return np.concatenate([res.results[k]["out"] for k in range(NCORES)], axis=0)


# revision 8
# speedup vs baseline: 1.0187x; 1.0187x over previous
"""HTSK fuzzy-system kernel for Trainium2 (Bass/Tile), 8-core data-parallel.

Math (per batch row b):
  S     = H/sigma^2 + EPS                          (D,R)
  m     = mean_d(-(X_bd - C_dr)^2 * S_dr)          (B,R)
        = X^2 @ (-S/D) + X @ (2*S*C/D) + K2        (matmul expansion)
  e     = exp(m)            (unnormalized softmax; m is bounded, no max needed)
  out   = (1/sum_r e) * ( sum_r e_br * G_bro  +  e @ (W2 + 1 b^T) )
  G     = X @ Wt,  Wt[d, h*4096 + o*64 + rr] = W[(h*64+rr)*D+d, o]

G layout: r split in low/high halves (h), o-major within each, rr innermost.
- innermost rr is step-1 so the e-broadcast multiply runs in DVE 2x_1P mode
- the r-halves live in two separate contiguous 4KB-per-partition tiles, so
  tree level 1 is ONE contiguous SBUF->SBUF DMA with the CCE inline adder
  (accum_op=add), running on the otherwise-idle DMA engines
When sigmas are uniform the X^2 term is constant over r and cancels in the
softmax, so the A-matmuls are dropped. sum_r e comes from a ones column
appended to W2.

Two phases per core:
  1) logits/exp/e-transpose/out2 for all 4 row-tiles, using 3 PSUM banks
     (scoped pools), overlapped with the Wt DMA stream
  2) G phase: all 8 PSUM banks as 2x[128,2048] fp32 ping-pong; per
     pair-chunk 8 matmuls (stationary changes once), ScalarE drain to bf16,
     DVE 2x multiply; tree L1 on DMA (CCE), L2..L7 + epilogue on DVE

Sharding: batch B=4096 split 512 rows per core; weights replicated.
All small constants + X^T ship in one packed [128, x] DMA blob per core.
"""
import os
import sys
import types

import numpy as np

sys.path.insert(0, "/opt/trn_rl_repo")

# NTFF profile-hook registry: trn_boot sets it at jax init, concourse
# bass_utils reads it when trace=True. The container's antenv package lacks
# this submodule, so provide it before anything imports jax/concourse.
if "antenv.axon_hooks" not in sys.modules:
    _ah = types.ModuleType("antenv.axon_hooks")
    _ah._hook = None

    def _set_hook(hook):
        _ah._hook = hook

    def _get_hook():
        return _ah._hook

    _ah.set_axon_ntff_profile_hook = _set_hook
    _ah.get_axon_ntff_profile_hook = _get_hook
    sys.modules["antenv.axon_hooks"] = _ah

import ml_dtypes  # noqa: E402
import concourse.bass as bass  # noqa: E402
import concourse.bacc as bacc  # noqa: E402
import concourse.tile as tile  # noqa: E402
from contextlib import ExitStack  # noqa: E402
from concourse import mybir  # noqa: E402
from concourse import bass_utils  # noqa: E402
from concourse.masks import make_identity  # noqa: E402

H = 0.5
EPS = 1e-8
B, D, R, O = 4096, 256, 128, 64
NCORES = 8
BL = B // NCORES          # 512 batch rows per core
NT = BL // 128            # 4 partition tiles per core
RO = R * O                # 8192 G columns per row
F32 = mybir.dt.float32
BF16 = mybir.dt.bfloat16
EXPF = mybir.ActivationFunctionType.Exp
ADD = mybir.AluOpType.add

# const blob column offsets (bf16 elements)
OBM, OK2, OW2, OXT = 0, 256, 384, 520
NC_BASE = OXT + 1024          # 1544
OA = NC_BASE                  # A appended when sigmas are non-uniform

_CACHE = {}
LAST_RESULT = None
TRACE = False
TRACE_DIR = "/root/problem/work/trace_out"
L1_DMA = os.environ.get("L1_DMA", "1") == "1"   # level-1 tree on SDMA vs DVE


def _build(use_a: bool):
    ncols = NC_BASE + (256 if use_a else 0)
    nc = bacc.Bacc("TRN2", target_bir_lowering=False, debug=False)
    CONST = nc.dram_tensor("CONST", [128, ncols], BF16, kind="ExternalInput")
    WT = nc.dram_tensor("WT", [D, RO], BF16, kind="ExternalInput")
    out = nc.dram_tensor("out", [BL, O], F32, kind="ExternalOutput")

    with tile.TileContext(nc) as tc, ExitStack() as ctx:
        consts = ctx.enter_context(tc.tile_pool(name="consts", bufs=1))
        work = ctx.enter_context(tc.tile_pool(name="work", bufs=2))
        gsbp = ctx.enter_context(tc.tile_pool(name="gsb", bufs=2))
        gap = ctx.enter_context(tc.tile_pool(name="ga", bufs=2))
        gbp = ctx.enter_context(tc.tile_pool(name="gb", bufs=2))
        treep = ctx.enter_context(tc.tile_pool(name="tree", bufs=2))

        # ---- one packed const DMA; Wt streams on two queues behind it ----
        conc = consts.tile([128, ncols], BF16, tag="conc")
        nc.sync.dma_start(out=conc[:, :], in_=CONST[:, :])
        bm_sb = conc[:, OBM:OBM + 256].rearrange("p (c r) -> p c r", r=R)
        k2_sb = conc[0:1, OK2:OK2 + R]
        w2b_sb = conc[:, OW2:OW2 + O + 1]          # [R, 65]: W2+b ++ ones col
        xTv = conc[:, OXT:OXT + 1024].rearrange("p (c j) -> p c j", j=BL)
        if use_a:
            a_sb = conc[:, OA:OA + 256].rearrange("p (c r) -> p c r", r=R)
        identB = consts.tile([128, 128], BF16, tag="idb")
        make_identity(nc, identB)
        ones_sb = consts.tile([1, 128], BF16, tag="ones")
        nc.vector.memset(ones_sb, 1.0)
        # Wt pair-chunk tiles: c0 on sync ring, c1 on gpsimd (SWDGE) ring
        wt_sb = [[None] * 4, [None] * 4]
        for q in range(4):
            for c in range(2):
                t_ = consts.tile([128, 2048], BF16, tag=f"wt{c}{q}")
                eng = nc.sync if c == 0 else nc.gpsimd
                eng.dma_start(out=t_[:, :],
                              in_=WT[c * 128:(c + 1) * 128,
                                     q * 2048:(q + 1) * 2048])
                wt_sb[c][q] = t_
        if use_a:
            x2T = consts.tile([128, 2, BL], BF16, tag="x2T")
            for c in range(2):
                nc.scalar.square(x2T[:, c, :], xTv[:, c, :])

        # per-tile softmax state, alive through phase 2
        e_bf = [consts.tile([128, R], BF16, tag=f"e{t}", name=f"e_{t}")
                for t in range(NT)]
        rs_all = consts.tile([128, NT], F32, tag="rs")
        o2_sb = consts.tile([128, NT * (O + 1)], F32, tag="o2sb")

        # ---- phase 1: logits / exp / e^T / out2 for all tiles ----
        with tc.tile_pool(name="ps_pre", bufs=1, space="PSUM") as ps_pre, \
             tc.tile_pool(name="ps_eT", bufs=2, space="PSUM") as ps_eT:
            m_ps = ps_pre.tile([128, NT * R], F32, tag="m")       # 1 bank
            o2_ps = ps_pre.tile([128, NT * (O + 1)], F32, tag="o2")  # 1 bank
            for t in range(NT):
                bs = slice(t * 128, (t + 1) * 128)
                mt = m_ps[:, t * R:(t + 1) * R]
                first = True
                if use_a:
                    for c in range(2):
                        nc.tensor.matmul(mt, lhsT=x2T[:, c, bs],
                                         rhs=a_sb[:, c, :],
                                         start=first, stop=False)
                        first = False
                for c in range(2):
                    nc.tensor.matmul(mt, lhsT=xTv[:, c, bs], rhs=bm_sb[:, c, :],
                                     start=first, stop=False)
                    first = False
                nc.tensor.matmul(mt, lhsT=ones_sb, rhs=k2_sb,
                                 start=False, stop=True)
            for t in range(NT):
                nc.scalar.activation(e_bf[t], m_ps[:, t * R:(t + 1) * R],
                                     EXPF, bias=0.0, scale=1.0)
            for t in range(NT):
                eT_ps = ps_eT.tile([128, 128], BF16, tag="eT")
                nc.tensor.transpose(eT_ps, e_bf[t], identB)
                eT_sb = work.tile([128, 128], BF16, tag="eTsb")
                nc.vector.tensor_copy(eT_sb, eT_ps)
                nc.tensor.matmul(o2_ps[:, t * 65:(t + 1) * 65], lhsT=eT_sb,
                                 rhs=w2b_sb, start=True, stop=True)
            nc.vector.tensor_copy(o2_sb, o2_ps)
            for t in range(NT):
                nc.vector.reciprocal(rs_all[:, t:t + 1],
                                     o2_sb[:, t * 65 + O:t * 65 + O + 1])

        # ---- phase 2: G matmuls, drains, multiplies, tree ----
        with tc.tile_pool(name="ps_g", bufs=2, space="PSUM") as ps_g:
            for t in range(NT):
                bs = slice(t * 128, (t + 1) * 128)
                gsb = gsbp.tile([128, RO], BF16, tag="gsb")
                ga = gap.tile([128, 4096], BF16, tag="ga")
                gb = gbp.tile([128, 4096], BF16, tag="gb")
                ga3 = ga.rearrange("p (o r) -> p o r", r=64)
                gb3 = gb.rearrange("p (o r) -> p o r", r=64)
                gsb3 = gsb.rearrange("p (o r) -> p o r", r=64)

                for pq in range(4):
                    gt = ps_g.tile([128, 2048], F32, tag="g", name=f"g_{t}_{pq}")
                    for c in range(2):
                        for h in range(4):
                            mm = nc.tensor.matmul(
                                gt[:, h * 512:(h + 1) * 512],
                                lhsT=xTv[:, c, bs],
                                rhs=wt_sb[c][pq][:, h * 512:(h + 1) * 512],
                                start=(c == 0), stop=(c == 1),
                            )
                            if h:
                                mm.ins.ldweights = False
                    nc.scalar.copy(gsb[:, pq * 2048:(pq + 1) * 2048], gt)
                    half, oq = divmod(pq, 2)
                    dst3 = (ga3 if half == 0 else gb3)[:, oq * 32:(oq + 1) * 32, :]
                    ebc = (e_bf[t][:, half * 64:(half + 1) * 64]
                           .rearrange("p r -> p () r").broadcast_to((128, 32, 64)))
                    nc.vector.tensor_mul(
                        dst3, gsb3[:, pq * 32:(pq + 1) * 32, :], ebc)

                # tree level 1: ga += gb, contiguous 4KB/partition streams
                if L1_DMA and t < NT - 1:
                    nc.gpsimd.dma_start(out=ga[:, :], in_=gb[:, :], accum_op=ADD)
                    z = ga3
                else:
                    t1 = treep.tile([128, 64, 64], BF16, tag="t1")
                    nc.vector.tensor_add(t1, ga3, gb3)
                    z = t1
                t2 = treep.tile([128, 64, 32], BF16, tag="t2")
                nc.vector.tensor_add(t2, z[:, :, 0:32], z[:, :, 32:64])
                t3 = treep.tile([128, 64, 16], BF16, tag="t3")
                nc.vector.tensor_add(t3, t2[:, :, 0:16], t2[:, :, 16:32])
                t4 = treep.tile([128, 64, 8], BF16, tag="t4")
                nc.vector.tensor_add(t4, t3[:, :, 0:8], t3[:, :, 8:16])
                t5 = treep.tile([128, 64, 4], BF16, tag="t5")
                nc.vector.tensor_add(t5, t4[:, :, 0:4], t4[:, :, 4:8])
                t6 = treep.tile([128, 64, 2], BF16, tag="t6")
                nc.vector.tensor_add(t6, t5[:, :, 0:2], t5[:, :, 2:4])
                red = work.tile([128, 64, 1], BF16, tag="red")
                nc.vector.tensor_add(red, t6[:, :, 0:1], t6[:, :, 1:2])

                # out = rs * (red + out2)
                osb = work.tile([128, O], F32, tag="osb")
                nc.vector.tensor_add(osb, red.rearrange("p o () -> p o"),
                                     o2_sb[:, t * 65:t * 65 + O])
                nc.vector.tensor_scalar_mul(osb, osb, rs_all[:, t:t + 1])
                nc.sync.dma_start(out=out[t * 128:(t + 1) * 128, :], in_=osb)

    nc.finalize()
    return nc


def _get_nc(use_a: bool):
    key = ("nc", use_a)
    if key not in _CACHE:
        _CACHE[key] = _build(use_a)
    return _CACHE[key]


def _host_prep(centers, sigmas, W, b):
    c64 = centers.astype(np.float64)
    S = (H / sigmas.astype(np.float64) ** 2) + EPS          # (D,R)
    use_a = not np.allclose(S, S.flat[0])
    bf = ml_dtypes.bfloat16
    ncols = NC_BASE + (256 if use_a else 0)
    CB = np.zeros((128, ncols), dtype=bf)
    Bm = (2.0 * S * c64 / D).astype(bf)                      # X coeff
    CB[:, OBM:OBM + 128] = Bm[0:128]
    CB[:, OBM + 128:OBM + 256] = Bm[128:256]
    K2 = (-(S * c64 * c64).sum(axis=0) / D).astype(bf)
    CB[0, OK2:OK2 + R] = K2
    W2b = np.concatenate(
        [W[D * R:].astype(np.float64) + b[None, :].astype(np.float64),
         np.ones((R, 1))], axis=1
    ).astype(bf)
    CB[:, OW2:OW2 + O + 1] = W2b
    if use_a:
        A = (-S / D).astype(bf)
        CB[:, OA:OA + 128] = A[0:128]
        CB[:, OA + 128:OA + 256] = A[128:256]
    W1 = W[: D * R].reshape(2, 64, D, O)          # (half, rr, d, o)
    # split-half o-major: Wt[d, h*4096 + o*64 + rr] = W1[h, rr, d, o]
    Wt = np.ascontiguousarray(
        W1.transpose(2, 0, 3, 1).reshape(D, RO)).astype(bf)
    return use_a, CB, Wt


def kernel(X, centers, sigmas, W, b):
    global LAST_RESULT
    X = np.asarray(X, dtype=np.float32)
    centers = np.asarray(centers, dtype=np.float32)
    sigmas = np.asarray(sigmas, dtype=np.float32)
    W = np.asarray(W, dtype=np.float32)
    b = np.asarray(b, dtype=np.float32)

    use_a, CB, Wt = _host_prep(centers, sigmas, W, b)
    Xb = X.astype(ml_dtypes.bfloat16)
    nc = _get_nc(use_a)
    in_maps = []
    for k in range(NCORES):
        cb = CB.copy()
        xt = Xb[k * BL:(k + 1) * BL].T                      # (D, BL)
        cb[:, OXT:OXT + BL] = xt[0:128]
        cb[:, OXT + BL:OXT + 1024] = xt[128:256]
        in_maps.append({"CONST": cb, "WT": Wt})
    kw = {}
    if TRACE:
        import shutil
        shutil.rmtree(TRACE_DIR, ignore_errors=True)
        kw = {"trace": True, "tmpdir": TRACE_DIR}
    res = bass_utils.run_bass_kernel_spmd(
        nc, in_maps, core_ids=list(range(NCORES)), **kw
    )
    LAST_RESULT = res
    return np.concatenate([res.results[k]["out"] for k in range(NCORES)], axis=0)
